# revision 1
# baseline (speedup 1.0000x reference)
"""EntropyGate fused kernel for 8 Trainium2 NeuronCores.

Problem (hardcoded shapes): B=4, S=4096, D=2048, window=8.
  H = entropy of softmax over sliding causal window (8) of token L2 norms of x
  gate_in = [y_ssm | y_attn | H]  (B,S,2D+1)
  h = silu(gate_in @ W1 + b1); g = sigmoid(h @ W2 + b2)
  out = g*y_ssm + (1-g)*y_attn

Sharding: flatten tokens (B*S = 16384) -> 8 shards of 2048 tokens (each shard
lies within one sequence; halo of 7 previous tokens of x for the entropy
window, zeros at sequence starts). Gate MLP weights replicated.

Device layout: feature-major ("transposed") activations so the contraction
dim (features) lands on SBUF partitions. Host supplies y_ssm/y_attn shards
pre-transposed (bf16 for matmul + f32 copy for the final gating); output is
produced transposed [D, tok] and transposed back on host.
"""

import numpy as np
import ml_dtypes

P = 128
D = 2048
TOK = 2048        # tokens per core
HALF = 1024       # token half processed per pass
NT = 512          # psum n-tile (fp32 PSUM bank limit)
MT = 16           # d_out tiles of 128
KC = 32           # 128-row feature chunks of [yT_ssm; yT_attn]
K2 = 16           # contraction chunks for mm2
WIN = 8
EXT = TOK + WIN - 1   # 2055
N_CORES = 8
B, S = 4, 4096

_BF16 = ml_dtypes.bfloat16
_NC_CACHE = {}


def _build_nc():
    import concourse.bass as bass
    import concourse.tile as tile
    import concourse.mybir as mybir
    from concourse import bacc
    from contextlib import ExitStack

    f32 = mybir.dt.float32
    bf16 = mybir.dt.bfloat16
    AF = mybir.ActivationFunctionType
    AX = mybir.AxisListType
    ALU = mybir.AluOpType

    nc = bacc.Bacc("TRN2", target_bir_lowering=False, debug=False, num_devices=1)

    yt16 = nc.dram_tensor("yt16", [2 * D, TOK], bf16, kind="ExternalInput")
    yf = nc.dram_tensor("yf", [2 * D, TOK], f32, kind="ExternalInput")
    xh = nc.dram_tensor("xh", [EXT, D], bf16, kind="ExternalInput")
    w1 = nc.dram_tensor("w1", [2 * D + 1, D], bf16, kind="ExternalInput")
    w2 = nc.dram_tensor("w2", [D, D], bf16, kind="ExternalInput")
    b1v = nc.dram_tensor("b1v", [D], f32, kind="ExternalInput")
    b2v = nc.dram_tensor("b2v", [D], f32, kind="ExternalInput")
    outT = nc.dram_tensor("outT", [D, TOK], f32, kind="ExternalOutput")
    # per-token-half entropy scratch (separate tensors keep the two entropy
    # pipelines independent in the dependency tracker)
    m_scr = [nc.dram_tensor(f"m_scr{i}", [9 * P], f32, kind="Internal")
             for i in range(2)]
    h_scr = [nc.dram_tensor(f"h_scr{i}", [HALF], bf16, kind="Internal")
             for i in range(2)]

    with tile.TileContext(nc) as tc:
        with ExitStack() as ctx:
            ent = ctx.enter_context(tc.tile_pool(name="ent", bufs=2))
            smol = ctx.enter_context(tc.tile_pool(name="smol", bufs=2))
            const = ctx.enter_context(tc.tile_pool(name="const", bufs=1))
            gate = ctx.enter_context(tc.tile_pool(name="gate", bufs=34))
            htp = ctx.enter_context(tc.tile_pool(name="htp", bufs=17))
            w1p = ctx.enter_context(tc.tile_pool(name="w1p", bufs=12))
            w2p = ctx.enter_context(tc.tile_pool(name="w2p", bufs=6))
            yfp = ctx.enter_context(tc.tile_pool(name="yfp", bufs=3))
            gp = ctx.enter_context(tc.tile_pool(name="gp", bufs=4))
            tp = ctx.enter_context(tc.tile_pool(name="tp", bufs=3))
            op = ctx.enter_context(tc.tile_pool(name="op", bufs=4))
            ps = ctx.enter_context(tc.tile_pool(name="ps", bufs=8, space="PSUM"))

            # ---- biases (per-partition columns: b[p, m] = b[m*128 + p]) ----
            b1sb = const.tile([P, MT], f32)
            nc.gpsimd.dma_start(b1sb[:], bass.AP(b1v, 0, [[1, P], [P, MT]]))
            b2sb = const.tile([P, MT], f32)
            nc.gpsimd.dma_start(b2sb[:], bass.AP(b2v, 0, [[1, P], [P, MT]]))
            negC = const.tile([P, 1], f32)
            nc.vector.memset(negC[:], -45.0)

            # one entropy pipeline per token-half; pipeline hh covers shard
            # tokens [hh*1024, hh*1024+1024) and consumes ext-row tiles
            # 8*hh .. 8*hh+8 (tile 8 is shared and squared twice).
            mcols = [const.tile([P, 9], f32, name="mcol", tag=f"mcol{i}")
                     for i in range(2)]
            nc.vector.memset(mcols[0][:], 1.0)
            nc.vector.memset(mcols[1][:], 1.0)

            def square_into(xt, rows, dst, use_act):
                if use_act:
                    nc.scalar.activation(
                        xt[:rows, :], xt[:rows, :], AF.Square,
                        accum_out=dst,
                    )
                else:
                    nc.vector.scalar_tensor_tensor(
                        xt[:rows, :], xt[:rows, :], 1.0, xt[:rows, :],
                        op0=ALU.mult, op1=ALU.mult,
                        accum_out=dst,
                    )

            def entropy_chain(hh):
                # norms: m = sqrt(s), one Newton step (ACT sqrt table is coarse)
                mc = mcols[hh]
                y0 = smol.tile([P, 9], f32, name="y0", tag=f"y0{hh}")
                nc.scalar.sqrt(y0[:], mc[:])
                y0e = smol.tile([P, 9], f32, name="y0e", tag=f"y0e{hh}")
                nc.vector.tensor_scalar_add(y0e[:], y0[:], 1e-30)
                rcp = smol.tile([P, 9], f32, name="rcp", tag=f"rcp{hh}")
                nc.vector.reciprocal(rcp[:], y0e[:])
                qt = smol.tile([P, 9], f32, name="qt", tag=f"qt{hh}")
                nc.vector.tensor_mul(qt[:], mc[:], rcp[:])
                msum = smol.tile([P, 9], f32, name="msum", tag=f"msum{hh}")
                nc.vector.tensor_add(msum[:], y0[:], qt[:])
                mf = smol.tile([P, 9], f32, name="mf", tag=f"mf{hh}")
                nc.scalar.mul(mf[:], msum[:], 0.5)
                nc.gpsimd.dma_start(bass.AP(m_scr[hh], 0, [[1, P], [P, 9]]), mf[:])
                # windows: wt[p, f, j] = m_ext[hh*1024 + p*16 + f + j]
                wt = smol.tile([64, 16, WIN], f32, name="wt", tag=f"wt{hh}")
                nc.gpsimd.dma_start(
                    wt[:], bass.AP(m_scr[hh], 0, [[16, 64], [1, 16], [1, WIN]])
                )
                et = smol.tile([64, 16, WIN], f32, name="et", tag=f"et{hh}")
                nc.scalar.activation(et[:], wt[:], AF.Exp, bias=negC[:64])
                pw = smol.tile([64, 16, WIN], f32, name="pw", tag=f"pw{hh}")
                nc.vector.tensor_mul(pw[:], et[:], wt[:])
                S_ = smol.tile([64, 16], f32, name="S_", tag=f"S{hh}")
                nc.vector.reduce_sum(S_[:], et[:], axis=AX.X)
                T_ = smol.tile([64, 16], f32, name="T_", tag=f"T{hh}")
                nc.vector.reduce_sum(T_[:], pw[:], axis=AX.X)
                R_ = smol.tile([64, 16], f32, name="R_", tag=f"R{hh}")
                nc.vector.reciprocal(R_[:], S_[:])
                L_ = smol.tile([64, 16], f32, name="L_", tag=f"L{hh}")
                nc.scalar.activation(L_[:], S_[:], AF.Ln)
                U_ = smol.tile([64, 16], f32, name="U_", tag=f"U{hh}")
                nc.vector.tensor_mul(U_[:], T_[:], R_[:])
                V_ = smol.tile([64, 16], f32, name="V_", tag=f"V{hh}")
                nc.vector.tensor_sub(V_[:], L_[:], U_[:])
                Hb = smol.tile([64, 16], bf16, name="Hb", tag=f"Hb{hh}")
                nc.vector.tensor_scalar(
                    Hb[:], V_[:], 45.0, 1.4426950408889634,
                    op0=ALU.add, op1=ALU.mult,
                )
                nc.gpsimd.dma_start(bass.AP(h_scr[hh], 0, [[16, 64], [1, 16]]), Hb[:])

            # ---- prologue: interleave half-0 gate chunks, first-mg W1 chunks
            # and entropy x tiles so PE starts mm1 asap while x streams in ----
            gts_half0 = []
            w1pre = []
            for k in range(KC):
                gt = gate.tile([P, HALF], bf16, name="gt", tag="gt")
                nc.sync.dma_start(gt[:], yt16.ap()[k * P:(k + 1) * P, 0:HALF])
                gts_half0.append(gt)
                if k < 10:
                    wp = w1p.tile([P, 4 * P], bf16, name="wtile", tag="w1t")
                    nc.sync.dma_start(wp[:], w1.ap()[k * P:(k + 1) * P, 0:512])
                    w1pre.append(wp)
                if k >= 2 and k % 2 == 0 and (k - 2) // 2 <= 8:
                    i = (k - 2) // 2
                    xt = ent.tile([P, D], bf16, name="xt", tag="xt")
                    nc.sync.dma_start(xt[:, :], xh.ap()[i * P:(i + 1) * P, :])
                    if i < 8:
                        square_into(xt, P, mcols[0][:, i:i + 1], i % 2 == 0)
                    else:
                        square_into(xt, P, mcols[0][:, 8:9], True)
                        nc.vector.tensor_copy(mcols[1][:, 0:1], mcols[0][:, 8:9])
                        entropy_chain(0)

            def emit_x_tail():
                # x ext-row tiles 9..16 — feed only half-1's entropy, which
                # isn't needed until half-1 mm1 (~380us): emit after mg0's
                # W1 stream so they don't starve the front DMA window.
                for i in range(9, 17):
                    rows = P if i < 16 else EXT - 16 * P
                    xt = ent.tile([P, D], bf16, name="xt", tag="xt")
                    nc.sync.dma_start(xt[:rows, :], xh.ap()[i * P:i * P + rows, :])
                    square_into(xt, rows, mcols[1][:rows, i - 8:i - 7], i % 2 == 0)
                entropy_chain(1)

            # ---- main: two token-halves ----
            gts_by_half = {0: gts_half0}
            for h in range(2):
                csl = slice(h * HALF, (h + 1) * HALF)
                gts = gts_by_half[h]
                hrow = const.tile([1, HALF], bf16, name="hrow", tag=f"hrow{h}")
                nc.gpsimd.dma_start(
                    hrow[:], bass.AP(h_scr[h], 0, [[HALF, 1], [1, HALF]])
                )

                hts = [htp.tile([P, HALF], bf16, name="ht", tag="ht")
                       for _ in range(MT)]

                # mm1: hT[m, tok] = silu(sum_k W1[k,m].T @ gateT[k,tok] + b1)
                gts_next = []
                for mg in range(4):
                    pts = [[ps.tile([P, NT], f32, name="pt1", tag="pt")
                            for _ in range(2)] for _ in range(4)]
                    wH = w1p.tile([1, 4 * P], bf16, name="wH", tag="wH", bufs=2)
                    nc.sync.dma_start(
                        wH[:], w1.ap()[2 * D:2 * D + 1, mg * 512:(mg + 1) * 512]
                    )
                    for k in range(KC):
                        if h == 0 and mg == 0 and k < len(w1pre):
                            wtile = w1pre[k]
                        else:
                            wtile = w1p.tile([P, 4 * P], bf16, name="wtile",
                                             tag="w1t")
                            nc.sync.dma_start(
                                wtile[:], w1.ap()[k * P:(k + 1) * P,
                                                  mg * 512:(mg + 1) * 512]
                            )
                        for mi in range(4):
                            for n in range(2):
                                nc.tensor.matmul(
                                    pts[mi][n][:],
                                    wtile[:, mi * P:(mi + 1) * P],
                                    gts[k][:, n * NT:(n + 1) * NT],
                                    start=(k == 0), stop=False,
                                )
                        if h == 0 and mg == 3:
                            gt = gate.tile([P, HALF], bf16, name="gt", tag="gt")
                            nc.sync.dma_start(
                                gt[:], yt16.ap()[k * P:(k + 1) * P, HALF:2 * HALF]
                            )
                            gts_next.append(gt)

                    if h == 0 and mg == 0:
                        emit_x_tail()
                    for mi in range(4):
                        m = mg * 4 + mi
                        for n in range(2):
                            nc.tensor.matmul(
                                pts[mi][n][:],
                                wH[:, mi * P:(mi + 1) * P],
                                hrow[:, n * NT:(n + 1) * NT],
                                start=False, stop=True,
                            )
                            nc.scalar.activation(
                                hts[m][:, n * NT:(n + 1) * NT], pts[mi][n][:],
                                AF.Silu, bias=b1sb[:, m:m + 1],
                            )

                if h == 0:
                    gts_by_half[1] = gts_next

                # mm2 + sigmoid + gating (small trailing groups cut the tail)
                # prefetch the last group's W2 tiles: late in the mm2 window
                # the DMA queues are saturated with yf/out traffic
                w2pre = []
                for k2 in range(K2):
                    wpre = w2p.tile([P, 2 * P], bf16, name="w2pre", tag="w2s",
                                    bufs=17)
                    nc.sync.dma_start(
                        wpre[:], w2.ap()[k2 * P:(k2 + 1) * P, 14 * P:16 * P]
                    )
                    w2pre.append(wpre)
                e_groups = [[0, 1, 2, 3], [4, 5, 6, 7], [8, 9, 10, 11],
                            [12, 13], [14, 15]]
                for egrp in e_groups:
                    ng = len(egrp)
                    pts2 = [[ps.tile([P, NT], f32, name="pt2", tag="pt")
                             for _ in range(2)] for _ in range(ng)]
                    for k2 in range(K2):
                        if egrp[0] == 14:
                            wtile2 = w2pre[k2]
                        else:
                            wtile2 = w2p.tile([P, ng * P], bf16, name="wtile2",
                                              tag="w2t")
                            nc.sync.dma_start(
                                wtile2[:], w2.ap()[k2 * P:(k2 + 1) * P,
                                                   egrp[0] * P:(egrp[-1] + 1) * P]
                            )
                        for ei in range(ng):
                            for n in range(2):
                                nc.tensor.matmul(
                                    pts2[ei][n][:],
                                    wtile2[:, ei * P:(ei + 1) * P],
                                    hts[k2][:, n * NT:(n + 1) * NT],
                                    start=(k2 == 0), stop=(k2 == K2 - 1),
                                )
                    for ei in range(ng):
                        e = egrp[ei]
                        ysf = yfp.tile([P, HALF], f32, name="ysf", tag="ysf")
                        nc.sync.dma_start(ysf[:], yf.ap()[e * P:(e + 1) * P, csl])
                        yaf = yfp.tile([P, HALF], f32, name="yaf", tag="yaf")
                        nc.sync.dma_start(
                            yaf[:], yf.ap()[D + e * P:D + (e + 1) * P, csl]
                        )
                        for n in range(2):
                            nsl = slice(n * NT, (n + 1) * NT)
                            g = gp.tile([P, NT], f32, name="g", tag="g")
                            nc.scalar.activation(
                                g[:], pts2[ei][n][:], AF.Sigmoid,
                                bias=b2sb[:, e:e + 1],
                            )
                            dsub = tp.tile([P, NT], f32, name="dsub", tag="dsub")
                            nc.vector.tensor_sub(dsub[:], ysf[:, nsl], yaf[:, nsl])
                            prod = tp.tile([P, NT], f32, name="prod", tag="prod")
                            nc.vector.tensor_mul(prod[:], g[:], dsub[:])
                            ot = op.tile([P, NT], f32, name="ot", tag="ot")
                            nc.vector.tensor_add(ot[:], prod[:], yaf[:, nsl])
                            nc.sync.dma_start(
                                outT.ap()[e * P:(e + 1) * P,
                                          h * HALF + n * NT:h * HALF + (n + 1) * NT],
                                ot[:],
                            )
    nc.finalize()
    return nc


def _get_nc():
    if "nc" not in _NC_CACHE:
        _NC_CACHE["nc"] = _build_nc()
    return _NC_CACHE["nc"]


def _make_in_maps(y_ssm, y_attn, x, W1, b1, W2, b2):
    ys = np.ascontiguousarray(np.asarray(y_ssm, np.float32).reshape(-1, D))
    ya = np.ascontiguousarray(np.asarray(y_attn, np.float32).reshape(-1, D))
    xs = np.ascontiguousarray(np.asarray(x, np.float32).reshape(-1, D))
    w1_bf = np.asarray(W1, np.float32).astype(_BF16)
    w2_bf = np.asarray(W2, np.float32).astype(_BF16)
    b1f = np.ascontiguousarray(np.asarray(b1, np.float32))
    b2f = np.ascontiguousarray(np.asarray(b2, np.float32))

    in_maps = []
    for c in range(N_CORES):
        t0 = c * TOK
        ysT = np.ascontiguousarray(ys[t0:t0 + TOK].T)   # (D, TOK) f32
        yaT = np.ascontiguousarray(ya[t0:t0 + TOK].T)
        yt16 = np.empty((2 * D, TOK), _BF16)
        yt16[:D] = ysT
        yt16[D:] = yaT
        yfc = np.empty((2 * D, TOK), np.float32)
        yfc[:D] = ysT
        yfc[D:] = yaT
        xe = np.zeros((EXT, D), np.float32)
        if t0 % S != 0:
            xe[:WIN - 1] = xs[t0 - (WIN - 1):t0]
        xe[WIN - 1:] = xs[t0:t0 + TOK]
        in_maps.append({
            "yt16": yt16,
            "yf": yfc,
            "xh": xe.astype(_BF16),
            "w1": w1_bf,
            "w2": w2_bf,
            "b1v": b1f,
            "b2v": b2f,
        })
    return in_maps


def _run(in_maps, trace=False):
    from concourse.bass_utils import run_bass_kernel_spmd
    nc = _get_nc()
    return run_bass_kernel_spmd(
        nc, in_maps, core_ids=list(range(N_CORES)), trace=trace
    )


def kernel(y_ssm, y_attn, x, W1, b1, W2, b2):
    in_maps = _make_in_maps(y_ssm, y_attn, x, W1, b1, W2, b2)
    res = _run(in_maps, trace=False)
    shards = [np.ascontiguousarray(r["outT"].T) for r in res.results]  # (TOK, D)
    full = np.concatenate(shards, axis=0)  # (16384, D)
    return full.reshape(B, S, D).astype(np.float32)



# revision 5
# speedup vs baseline: 2.9847x; 2.9847x over previous
"""EntropyGate fused kernel for Trainium2 NeuronCores — transfer-optimized.

Problem (hardcoded shapes): B=4, S=4096, D=2048, window=8.
  H = entropy of softmax over sliding causal window (8) of token L2 norms of x
  gate_in = [y_ssm | y_attn | H]  (B,S,2D+1)
  h = silu(gate_in @ W1 + b1); g = sigmoid(h @ W2 + b2)
  out = g*y_ssm + (1-g)*y_attn

The axon tunnel to the cores moves ~35-40 MB/s, so wall time is transfer
bound; the design minimizes bytes on the wire:
  - y_ssm/y_attn are sent as per-token-scaled int8 (1 B/elem).
  - W1/W2 are sent as per-column-scaled int8; the column scales fold into
    the Silu/Sigmoid activation `scale` operand on device (exact).
  - The entropy feature H is computed on host (needs only token norms)
    and shipped as TOK floats per core.
  - The device returns only the gate g quantized to uint8; the host
    reconstructs out = ya + g*(ys-ya) from its full-precision inputs.
Device-side, activations arrive token-major and are PE-transposed to
feature-major for the matmuls (host transposes would dominate wall time).

Sharding: tokens (B*S = 16384) split evenly across cores; weights
replicated per core.
"""

import numpy as np
import ml_dtypes

P = 128
B, S, D = 4, 4096, 2048
T = B * S                 # total tokens
N_CORES = 8               # cores used (token shards)
TOK = T // N_CORES        # tokens per core
CH = 1024                 # token chunk processed per pass
NCH = TOK // CH
MT = D // P               # 16 output blocks of 128
KC = 2 * D // P           # 32 contraction tiles for mm1
K2 = D // P               # 16 contraction tiles for mm2
WIN = 8
W2R = 2 * D + 1           # row offset of W2 inside packed wq

# aux (f32) packing offsets, per core
A_YSC = 0
A_YAC = TOK
A_H = 2 * TOK
A_W1S = 3 * TOK
A_W2S = 3 * TOK + D
A_B1 = 3 * TOK + 2 * D
A_B2 = 3 * TOK + 3 * D
AUX = 3 * TOK + 4 * D

_BF16 = ml_dtypes.bfloat16
_NC_CACHE = {}


def _build_nc():
    import concourse.bass as bass
    import concourse.tile as tile
    import concourse.mybir as mybir
    from concourse import bacc
    from contextlib import ExitStack

    f32 = mybir.dt.float32
    bf16 = mybir.dt.bfloat16
    i8 = mybir.dt.int8
    u8 = mybir.dt.uint8
    i32 = mybir.dt.int32
    AF = mybir.ActivationFunctionType
    ALU = mybir.AluOpType

    nc = bacc.Bacc("TRN2", target_bir_lowering=False, debug=False, num_devices=1)

    yq = nc.dram_tensor("yq", [2 * TOK, D], i8, kind="ExternalInput")
    wq = nc.dram_tensor("wq", [W2R + D, D], i8, kind="ExternalInput")
    aux = nc.dram_tensor("aux", [AUX], f32, kind="ExternalInput")
    g8 = nc.dram_tensor("g8", [TOK, D], u8, kind="ExternalOutput")

    with tile.TileContext(nc) as tc:
        with ExitStack() as ctx:
            const = ctx.enter_context(tc.tile_pool(name="const", bufs=1))
            stage = ctx.enter_context(tc.tile_pool(name="stage", bufs=3))
            gatep = ctx.enter_context(tc.tile_pool(name="gatep", bufs=1))
            htp = ctx.enter_context(tc.tile_pool(name="htp", bufs=1))
            gbp = ctx.enter_context(tc.tile_pool(name="gbp", bufs=1))
            gnp = ctx.enter_context(tc.tile_pool(name="gnp", bufs=2))
            wp = ctx.enter_context(tc.tile_pool(name="wp", bufs=4))
            zp = ctx.enter_context(tc.tile_pool(name="zp", bufs=3))
            smol = ctx.enter_context(tc.tile_pool(name="smol", bufs=2))
            ps = ctx.enter_context(tc.tile_pool(name="ps", bufs=8, space="PSUM"))

            # 128x128 identity for PE transposes
            iot = const.tile([P, P], i32)
            nc.gpsimd.iota(iot[:], pattern=[[1, P]], channel_multiplier=-1)
            ident = const.tile([P, P], bf16)
            nc.vector.tensor_scalar(
                ident[:], iot[:], 0, 1.0, op0=ALU.is_equal, op1=ALU.mult
            )

            # per-partition scale/bias columns: col m holds values for
            # output block m (w1scb[p, m] = w1sc[m*128 + p], etc.)
            w1scb = const.tile([P, MT], f32)
            nc.gpsimd.dma_start(w1scb[:], bass.AP(aux, A_W1S, [[1, P], [P, MT]]))
            w2scb = const.tile([P, MT], f32)
            nc.gpsimd.dma_start(w2scb[:], bass.AP(aux, A_W2S, [[1, P], [P, MT]]))
            b1sb = const.tile([P, MT], f32)
            nc.gpsimd.dma_start(b1sb[:], bass.AP(aux, A_B1, [[1, P], [P, MT]]))
            b2sb = const.tile([P, MT], f32)
            nc.gpsimd.dma_start(b2sb[:], bass.AP(aux, A_B2, [[1, P], [P, MT]]))

            NB = CH // P   # 128-token blocks per chunk
            N2 = CH // 512  # psum n-splits per chunk

            for c in range(NCH):
                # ---- phase A: load int8 y, dequant, PE-transpose to
                # feature-major gateT tiles [128 feat, CH tok] ----
                gts = []
                for y in range(2):
                    row = [gatep.tile([P, CH], bf16, name="gt",
                                      tag=f"gt{y}_{f}") for f in range(MT)]
                    gts.append(row)
                    scoff = A_YSC if y == 0 else A_YAC
                    for r in range(NB):
                        row0 = y * TOK + c * CH + r * P
                        yt = stage.tile([P, D], i8, name="yt", tag="yt")
                        nc.sync.dma_start(yt[:], yq.ap()[row0:row0 + P, :])
                        sct = smol.tile([P, 1], f32, name="sct", tag="sct")
                        nc.gpsimd.dma_start(
                            sct[:],
                            bass.AP(aux, scoff + c * CH + r * P, [[1, P], [1, 1]]),
                        )
                        dq = stage.tile([P, D], bf16, name="dq", tag="dq")
                        nc.scalar.activation(dq[:], yt[:], AF.Copy,
                                             scale=sct[:, 0:1])
                        for f in range(MT):
                            pt = ps.tile([P, P], bf16, name="ptr", tag="pt")
                            nc.tensor.transpose(
                                pt[:], dq[:, f * P:(f + 1) * P], ident[:]
                            )
                            nc.vector.tensor_copy(
                                gts[y][f][:, r * P:(r + 1) * P], pt[:]
                            )
                gflat = gts[0] + gts[1]

                # H feature row for this chunk (f32 -> bf16 on device)
                hrf = smol.tile([1, CH], f32, name="hrf", tag="hrf")
                nc.gpsimd.dma_start(
                    hrf[:], bass.AP(aux, A_H + c * CH, [[CH, 1], [1, CH]])
                )
                hrow = smol.tile([1, CH], bf16, name="hrow", tag="hrow")
                nc.scalar.activation(hrow[:], hrf[:], AF.Copy)

                # ---- mm1: hT[m, tok] = silu(s1[m]*(W1raw.T @ gateT) + b1) ----
                hts = [htp.tile([P, CH], bf16, name="ht", tag=f"ht{m}")
                       for m in range(MT)]
                for mg in range(4):
                    csl = slice(mg * 512, (mg + 1) * 512)
                    pts = [[ps.tile([P, 512], f32, name="pt1", tag="pt")
                            for _ in range(N2)] for _ in range(4)]
                    wH8 = wp.tile([1, 512], i8, name="wH8", tag="wH8")
                    nc.sync.dma_start(wH8[:], wq.ap()[2 * D:2 * D + 1, csl])
                    wH = wp.tile([1, 512], bf16, name="wH", tag="wH")
                    nc.scalar.activation(wH[:], wH8[:], AF.Copy)
                    for k in range(KC):
                        w8 = wp.tile([P, 512], i8, name="w8", tag="w8")
                        nc.sync.dma_start(w8[:], wq.ap()[k * P:(k + 1) * P, csl])
                        wb = wp.tile([P, 512], bf16, name="wb", tag="wb")
                        nc.scalar.activation(wb[:], w8[:], AF.Copy)
                        for mi in range(4):
                            for n in range(N2):
                                nc.tensor.matmul(
                                    pts[mi][n][:],
                                    wb[:, mi * P:(mi + 1) * P],
                                    gflat[k][:, n * 512:(n + 1) * 512],
                                    start=(k == 0), stop=False,
                                )
                    for mi in range(4):
                        m = mg * 4 + mi
                        for n in range(N2):
                            nc.tensor.matmul(
                                pts[mi][n][:],
                                wH[:, mi * P:(mi + 1) * P],
                                hrow[:, n * 512:(n + 1) * 512],
                                start=False, stop=True,
                            )
                            # silu(z) = z * sigmoid(z), z = s1[m]*psum + b1[m]
                            zt = zp.tile([P, 512], f32, name="zt", tag="zt")
                            nc.scalar.activation(
                                zt[:], pts[mi][n][:], AF.Identity,
                                bias=b1sb[:, m:m + 1], scale=w1scb[:, m:m + 1],
                            )
                            sg = zp.tile([P, 512], f32, name="sg", tag="sg")
                            nc.scalar.activation(sg[:], zt[:], AF.Sigmoid)
                            nc.vector.tensor_mul(
                                hts[m][:, n * 512:(n + 1) * 512], zt[:], sg[:]
                            )

                # ---- mm2: gT[e, tok] = sigmoid(s2[e]*(W2raw.T @ hT) + b2) ----
                gbs = [gbp.tile([P, CH], bf16, name="gb", tag=f"gb{e}")
                       for e in range(MT)]
                for eg in range(4):
                    esl = slice(eg * 512, (eg + 1) * 512)
                    pts2 = [[ps.tile([P, 512], f32, name="pt2", tag="pt")
                             for _ in range(N2)] for _ in range(4)]
                    for k2 in range(K2):
                        w28 = wp.tile([P, 512], i8, name="w28", tag="w8")
                        nc.sync.dma_start(
                            w28[:], wq.ap()[W2R + k2 * P:W2R + (k2 + 1) * P, esl]
                        )
                        w2b = wp.tile([P, 512], bf16, name="w2b", tag="wb")
                        nc.scalar.activation(w2b[:], w28[:], AF.Copy)
                        for ei in range(4):
                            for n in range(N2):
                                nc.tensor.matmul(
                                    pts2[ei][n][:],
                                    w2b[:, ei * P:(ei + 1) * P],
                                    hts[k2][:, n * 512:(n + 1) * 512],
                                    start=(k2 == 0), stop=(k2 == K2 - 1),
                                )
                    for ei in range(4):
                        e = eg * 4 + ei
                        for n in range(N2):
                            nc.scalar.activation(
                                gbs[e][:, n * 512:(n + 1) * 512],
                                pts2[ei][n][:], AF.Sigmoid,
                                bias=b2sb[:, e:e + 1], scale=w2scb[:, e:e + 1],
                            )

                # ---- phase D: transpose g back to token-major, quantize
                # to u8 (conversion truncates, +0.5 rounds), store ----
                for r in range(NB):
                    gn = gnp.tile([P, D], u8, name="gn", tag="gn")
                    for e in range(MT):
                        ptg = ps.tile([P, P], bf16, name="ptg", tag="pt")
                        nc.tensor.transpose(
                            ptg[:], gbs[e][:, r * P:(r + 1) * P], ident[:]
                        )
                        nc.vector.tensor_scalar(
                            gn[:, e * P:(e + 1) * P], ptg[:], 255.0, 0.5,
                            op0=ALU.mult, op1=ALU.add,
                        )
                    nc.sync.dma_start(
                        g8.ap()[c * CH + r * P:c * CH + (r + 1) * P, :], gn[:]
                    )
    nc.finalize()
    return nc


def _get_nc():
    if "nc" not in _NC_CACHE:
        _NC_CACHE["nc"] = _build_nc()
    return _NC_CACHE["nc"]


def _entropy_host(x2d):
    # token L2 norms -> sliding causal window softmax entropy, (T,) f32
    m = np.sqrt(np.einsum("sd,sd->s", x2d, x2d)).reshape(B, S)
    off = np.arange(WIN) - (WIN - 1)
    idx = np.arange(S)[:, None] + off[None, :]
    valid = idx >= 0
    idxc = np.clip(idx, 0, S - 1)
    wins = m[:, idxc]
    wins = np.where(valid[None], wins, -np.inf)
    wmax = wins.max(-1, keepdims=True)
    e = np.exp(wins - wmax)
    p = e / e.sum(-1, keepdims=True)
    H = -(p * np.log2(p + 1e-9)).sum(-1)
    return np.ascontiguousarray(H.reshape(-1).astype(np.float32))


def _quant_rows_into(a, out_i8):
    # per-row symmetric int8: returns scales (rows,) f32
    hi = a.max(axis=1)
    lo = a.min(axis=1)
    s = np.maximum(hi, -lo)
    s /= 127.0
    np.maximum(s, 1e-30, out=s)
    inv = 1.0 / s
    tmp = a * inv[:, None]
    np.rint(tmp, out=tmp)
    out_i8[:] = tmp
    return s.astype(np.float32)


def _quant_cols(w):
    # per-column symmetric int8: returns (q, scales (cols,) f32)
    aw = np.abs(w).max(axis=0)
    s = np.maximum(aw / 127.0, 1e-30).astype(np.float32)
    tmp = w * (1.0 / s)[None, :]
    np.rint(tmp, out=tmp)
    return tmp.astype(np.int8), s


def _make_in_maps(y_ssm, y_attn, x, W1, b1, W2, b2):
    ys = np.asarray(y_ssm, np.float32).reshape(T, D)
    ya = np.asarray(y_attn, np.float32).reshape(T, D)
    xs = np.asarray(x, np.float32).reshape(T, D)
    W1f = np.asarray(W1, np.float32)
    W2f = np.asarray(W2, np.float32)
    b1f = np.asarray(b1, np.float32)
    b2f = np.asarray(b2, np.float32)

    Hent = _entropy_host(xs)

    w1q, w1s = _quant_cols(W1f)
    w2q, w2s = _quant_cols(W2f)
    wq = np.concatenate([w1q, w2q], axis=0)  # (2D+1+D, D): W2 rows start at W2R
    assert wq.shape[0] == W2R + D

    in_maps = []
    for c in range(N_CORES):
        t0 = c * TOK
        yq_c = np.empty((2 * TOK, D), np.int8)
        ysc = _quant_rows_into(ys[t0:t0 + TOK], yq_c[:TOK])
        yac = _quant_rows_into(ya[t0:t0 + TOK], yq_c[TOK:])
        aux = np.empty(AUX, np.float32)
        aux[A_YSC:A_YSC + TOK] = ysc
        aux[A_YAC:A_YAC + TOK] = yac
        aux[A_H:A_H + TOK] = Hent[t0:t0 + TOK]
        aux[A_W1S:A_W1S + D] = w1s
        aux[A_W2S:A_W2S + D] = w2s
        aux[A_B1:A_B1 + D] = b1f
        aux[A_B2:A_B2 + D] = b2f
        in_maps.append({"yq": yq_c, "wq": wq, "aux": aux})
    return in_maps, ys, ya


def _run(in_maps, trace=False):
    from concourse.bass_utils import run_bass_kernel_spmd
    nc = _get_nc()
    return run_bass_kernel_spmd(
        nc, in_maps, core_ids=list(range(N_CORES)), trace=trace
    )


def kernel(y_ssm, y_attn, x, W1, b1, W2, b2):
    in_maps, ys, ya = _make_in_maps(y_ssm, y_attn, x, W1, b1, W2, b2)
    res = _run(in_maps, trace=False)
    g = np.concatenate([r["g8"] for r in res.results], axis=0)  # (T, D) u8
    gf = g.astype(np.float32)
    gf *= np.float32(1.0 / 255.0)
    out = ys - ya
    out *= gf
    out += ya
    return out.reshape(B, S, D)


# revision 7
# speedup vs baseline: 3.0784x; 1.0314x over previous
"""EntropyGate fused kernel for Trainium2 NeuronCores — transfer-optimized.

Problem (hardcoded shapes): B=4, S=4096, D=2048, window=8.
  H = entropy of softmax over sliding causal window (8) of token L2 norms of x
  gate_in = [y_ssm | y_attn | H]  (B,S,2D+1)
  h = silu(gate_in @ W1 + b1); g = sigmoid(h @ W2 + b2)
  out = g*y_ssm + (1-g)*y_attn

The axon tunnel to the cores moves ~35-40 MB/s, so wall time is transfer
bound; the design minimizes bytes on the wire:
  - y_ssm/y_attn are sent as per-token-scaled int8 (1 B/elem).
  - W1/W2 are sent as per-column-scaled int8; the column scales fold into
    the Silu/Sigmoid activation `scale` operand on device (exact).
  - The entropy feature H is computed on host (needs only token norms)
    and shipped as TOK floats per core.
  - The device returns only the gate g quantized to uint8; the host
    reconstructs out = ya + g*(ys-ya) from its full-precision inputs.
Device-side, activations arrive token-major and are PE-transposed to
feature-major for the matmuls (host transposes would dominate wall time).

Sharding: tokens (B*S = 16384) split evenly across cores; weights
replicated per core.
"""

import numpy as np
import ml_dtypes

P = 128
B, S, D = 4, 4096, 2048
T = B * S                 # total tokens
N_CORES = 2               # cores used (token shards)
TOK = T // N_CORES        # tokens per core
CH = 1024                 # token chunk processed per pass
NCH = TOK // CH
MT = D // P               # 16 output blocks of 128
KC = 2 * D // P           # 32 contraction tiles for mm1
K2 = D // P               # 16 contraction tiles for mm2
WIN = 8
W2R = 2 * D + 1           # row offset of W2 inside packed wq

# aux (f32) packing offsets, per core
A_YSC = 0
A_YAC = TOK
A_H = 2 * TOK
A_W1S = 3 * TOK
A_W2S = 3 * TOK + D
A_B1 = 3 * TOK + 2 * D
A_B2 = 3 * TOK + 3 * D
AUX = 3 * TOK + 4 * D

_BF16 = ml_dtypes.bfloat16
_NC_CACHE = {}


def _build_nc():
    import concourse.bass as bass
    import concourse.tile as tile
    import concourse.mybir as mybir
    from concourse import bacc
    from contextlib import ExitStack

    f32 = mybir.dt.float32
    bf16 = mybir.dt.bfloat16
    i8 = mybir.dt.int8
    u8 = mybir.dt.uint8
    i32 = mybir.dt.int32
    AF = mybir.ActivationFunctionType
    ALU = mybir.AluOpType

    nc = bacc.Bacc("TRN2", target_bir_lowering=False, debug=False, num_devices=1)

    yq = nc.dram_tensor("yq", [2 * TOK, D], i8, kind="ExternalInput")
    wq = nc.dram_tensor("wq", [W2R + D, D], i8, kind="ExternalInput")
    aux = nc.dram_tensor("aux", [AUX], f32, kind="ExternalInput")
    g8 = nc.dram_tensor("g8", [TOK, D], u8, kind="ExternalOutput")

    with tile.TileContext(nc) as tc:
        with ExitStack() as ctx:
            const = ctx.enter_context(tc.tile_pool(name="const", bufs=1))
            stage = ctx.enter_context(tc.tile_pool(name="stage", bufs=3))
            gatep = ctx.enter_context(tc.tile_pool(name="gatep", bufs=1))
            htp = ctx.enter_context(tc.tile_pool(name="htp", bufs=1))
            gbp = ctx.enter_context(tc.tile_pool(name="gbp", bufs=1))
            gnp = ctx.enter_context(tc.tile_pool(name="gnp", bufs=2))
            wp = ctx.enter_context(tc.tile_pool(name="wp", bufs=4))
            zp = ctx.enter_context(tc.tile_pool(name="zp", bufs=3))
            smol = ctx.enter_context(tc.tile_pool(name="smol", bufs=2))
            ps = ctx.enter_context(tc.tile_pool(name="ps", bufs=8, space="PSUM"))

            # 128x128 identity for PE transposes
            iot = const.tile([P, P], i32)
            nc.gpsimd.iota(iot[:], pattern=[[1, P]], channel_multiplier=-1)
            ident = const.tile([P, P], bf16)
            nc.vector.tensor_scalar(
                ident[:], iot[:], 0, 1.0, op0=ALU.is_equal, op1=ALU.mult
            )

            # per-partition scale/bias columns: col m holds values for
            # output block m (w1scb[p, m] = w1sc[m*128 + p], etc.)
            w1scb = const.tile([P, MT], f32)
            nc.gpsimd.dma_start(w1scb[:], bass.AP(aux, A_W1S, [[1, P], [P, MT]]))
            w2scb = const.tile([P, MT], f32)
            nc.gpsimd.dma_start(w2scb[:], bass.AP(aux, A_W2S, [[1, P], [P, MT]]))
            b1sb = const.tile([P, MT], f32)
            nc.gpsimd.dma_start(b1sb[:], bass.AP(aux, A_B1, [[1, P], [P, MT]]))
            b2sb = const.tile([P, MT], f32)
            nc.gpsimd.dma_start(b2sb[:], bass.AP(aux, A_B2, [[1, P], [P, MT]]))

            NB = CH // P   # 128-token blocks per chunk
            N2 = CH // 512  # psum n-splits per chunk

            for c in range(NCH):
                # ---- phase A: load int8 y, dequant, PE-transpose to
                # feature-major gateT tiles [128 feat, CH tok] ----
                gts = []
                for y in range(2):
                    row = [gatep.tile([P, CH], bf16, name="gt",
                                      tag=f"gt{y}_{f}") for f in range(MT)]
                    gts.append(row)
                    scoff = A_YSC if y == 0 else A_YAC
                    for r in range(NB):
                        row0 = y * TOK + c * CH + r * P
                        yt = stage.tile([P, D], i8, name="yt", tag="yt")
                        nc.sync.dma_start(yt[:], yq.ap()[row0:row0 + P, :])
                        sct = smol.tile([P, 1], f32, name="sct", tag="sct")
                        nc.gpsimd.dma_start(
                            sct[:],
                            bass.AP(aux, scoff + c * CH + r * P, [[1, P], [1, 1]]),
                        )
                        dq = stage.tile([P, D], bf16, name="dq", tag="dq")
                        nc.scalar.activation(dq[:], yt[:], AF.Copy,
                                             scale=sct[:, 0:1])
                        for f in range(MT):
                            pt = ps.tile([P, P], bf16, name="ptr", tag="pt")
                            nc.tensor.transpose(
                                pt[:], dq[:, f * P:(f + 1) * P], ident[:]
                            )
                            nc.vector.tensor_copy(
                                gts[y][f][:, r * P:(r + 1) * P], pt[:]
                            )
                gflat = gts[0] + gts[1]

                # H feature row for this chunk (f32 -> bf16 on device)
                hrf = smol.tile([1, CH], f32, name="hrf", tag="hrf")
                nc.gpsimd.dma_start(
                    hrf[:], bass.AP(aux, A_H + c * CH, [[CH, 1], [1, CH]])
                )
                hrow = smol.tile([1, CH], bf16, name="hrow", tag="hrow")
                nc.scalar.activation(hrow[:], hrf[:], AF.Copy)

                # ---- mm1: hT[m, tok] = silu(s1[m]*(W1raw.T @ gateT) + b1) ----
                hts = [htp.tile([P, CH], bf16, name="ht", tag=f"ht{m}")
                       for m in range(MT)]
                for mg in range(4):
                    csl = slice(mg * 512, (mg + 1) * 512)
                    pts = [[ps.tile([P, 512], f32, name="pt1", tag="pt")
                            for _ in range(N2)] for _ in range(4)]
                    wH8 = wp.tile([1, 512], i8, name="wH8", tag="wH8")
                    nc.sync.dma_start(wH8[:], wq.ap()[2 * D:2 * D + 1, csl])
                    wH = wp.tile([1, 512], bf16, name="wH", tag="wH")
                    nc.scalar.activation(wH[:], wH8[:], AF.Copy)
                    for k in range(KC):
                        w8 = wp.tile([P, 512], i8, name="w8", tag="w8")
                        nc.sync.dma_start(w8[:], wq.ap()[k * P:(k + 1) * P, csl])
                        wb = wp.tile([P, 512], bf16, name="wb", tag="wb")
                        nc.scalar.activation(wb[:], w8[:], AF.Copy)
                        for mi in range(4):
                            for n in range(N2):
                                nc.tensor.matmul(
                                    pts[mi][n][:],
                                    wb[:, mi * P:(mi + 1) * P],
                                    gflat[k][:, n * 512:(n + 1) * 512],
                                    start=(k == 0), stop=False,
                                )
                    for mi in range(4):
                        m = mg * 4 + mi
                        for n in range(N2):
                            nc.tensor.matmul(
                                pts[mi][n][:],
                                wH[:, mi * P:(mi + 1) * P],
                                hrow[:, n * 512:(n + 1) * 512],
                                start=False, stop=True,
                            )
                            # silu(z) = z * sigmoid(z), z = s1[m]*psum + b1[m]
                            zt = zp.tile([P, 512], f32, name="zt", tag="zt")
                            nc.scalar.activation(
                                zt[:], pts[mi][n][:], AF.Identity,
                                bias=b1sb[:, m:m + 1], scale=w1scb[:, m:m + 1],
                            )
                            sg = zp.tile([P, 512], f32, name="sg", tag="sg")
                            nc.scalar.activation(sg[:], zt[:], AF.Sigmoid)
                            nc.vector.tensor_mul(
                                hts[m][:, n * 512:(n + 1) * 512], zt[:], sg[:]
                            )

                # ---- mm2: gT[e, tok] = sigmoid(s2[e]*(W2raw.T @ hT) + b2) ----
                gbs = [gbp.tile([P, CH], bf16, name="gb", tag=f"gb{e}")
                       for e in range(MT)]
                for eg in range(4):
                    esl = slice(eg * 512, (eg + 1) * 512)
                    pts2 = [[ps.tile([P, 512], f32, name="pt2", tag="pt")
                             for _ in range(N2)] for _ in range(4)]
                    for k2 in range(K2):
                        w28 = wp.tile([P, 512], i8, name="w28", tag="w8")
                        nc.sync.dma_start(
                            w28[:], wq.ap()[W2R + k2 * P:W2R + (k2 + 1) * P, esl]
                        )
                        w2b = wp.tile([P, 512], bf16, name="w2b", tag="wb")
                        nc.scalar.activation(w2b[:], w28[:], AF.Copy)
                        for ei in range(4):
                            for n in range(N2):
                                nc.tensor.matmul(
                                    pts2[ei][n][:],
                                    w2b[:, ei * P:(ei + 1) * P],
                                    hts[k2][:, n * 512:(n + 1) * 512],
                                    start=(k2 == 0), stop=(k2 == K2 - 1),
                                )
                    for ei in range(4):
                        e = eg * 4 + ei
                        for n in range(N2):
                            nc.scalar.activation(
                                gbs[e][:, n * 512:(n + 1) * 512],
                                pts2[ei][n][:], AF.Sigmoid,
                                bias=b2sb[:, e:e + 1], scale=w2scb[:, e:e + 1],
                            )

                # ---- phase D: transpose g back to token-major, quantize
                # to u8 (conversion truncates, +0.5 rounds), store ----
                for r in range(NB):
                    gn = gnp.tile([P, D], u8, name="gn", tag="gn")
                    for e in range(MT):
                        ptg = ps.tile([P, P], bf16, name="ptg", tag="pt")
                        nc.tensor.transpose(
                            ptg[:], gbs[e][:, r * P:(r + 1) * P], ident[:]
                        )
                        nc.vector.tensor_scalar(
                            gn[:, e * P:(e + 1) * P], ptg[:], 255.0, 0.5,
                            op0=ALU.mult, op1=ALU.add,
                        )
                    nc.sync.dma_start(
                        g8.ap()[c * CH + r * P:c * CH + (r + 1) * P, :], gn[:]
                    )
    nc.finalize()
    return nc


def _get_nc():
    if "nc" not in _NC_CACHE:
        _NC_CACHE["nc"] = _build_nc()
    return _NC_CACHE["nc"]


def _entropy_host(x2d):
    # token L2 norms -> sliding causal window softmax entropy, (T,) f32
    m = np.sqrt(np.einsum("sd,sd->s", x2d, x2d)).reshape(B, S)
    off = np.arange(WIN) - (WIN - 1)
    idx = np.arange(S)[:, None] + off[None, :]
    valid = idx >= 0
    idxc = np.clip(idx, 0, S - 1)
    wins = m[:, idxc]
    wins = np.where(valid[None], wins, -np.inf)
    wmax = wins.max(-1, keepdims=True)
    e = np.exp(wins - wmax)
    p = e / e.sum(-1, keepdims=True)
    H = -(p * np.log2(p + 1e-9)).sum(-1)
    return np.ascontiguousarray(H.reshape(-1).astype(np.float32))


def _quant_rows_into(a, out_i8):
    # per-row symmetric int8: returns scales (rows,) f32
    hi = a.max(axis=1)
    lo = a.min(axis=1)
    s = np.maximum(hi, -lo)
    s /= 127.0
    np.maximum(s, 1e-30, out=s)
    inv = 1.0 / s
    tmp = a * inv[:, None]
    np.rint(tmp, out=tmp)
    out_i8[:] = tmp
    return s.astype(np.float32)


def _quant_cols(w):
    # per-column symmetric int8: returns (q, scales (cols,) f32)
    aw = np.abs(w).max(axis=0)
    s = np.maximum(aw / 127.0, 1e-30).astype(np.float32)
    tmp = w * (1.0 / s)[None, :]
    np.rint(tmp, out=tmp)
    return tmp.astype(np.int8), s


def _make_in_maps(y_ssm, y_attn, x, W1, b1, W2, b2):
    ys = np.asarray(y_ssm, np.float32).reshape(T, D)
    ya = np.asarray(y_attn, np.float32).reshape(T, D)
    xs = np.asarray(x, np.float32).reshape(T, D)
    W1f = np.asarray(W1, np.float32)
    W2f = np.asarray(W2, np.float32)
    b1f = np.asarray(b1, np.float32)
    b2f = np.asarray(b2, np.float32)

    Hent = _entropy_host(xs)

    w1q, w1s = _quant_cols(W1f)
    w2q, w2s = _quant_cols(W2f)
    wq = np.concatenate([w1q, w2q], axis=0)  # (2D+1+D, D): W2 rows start at W2R
    assert wq.shape[0] == W2R + D

    in_maps = []
    for c in range(N_CORES):
        t0 = c * TOK
        yq_c = np.empty((2 * TOK, D), np.int8)
        ysc = _quant_rows_into(ys[t0:t0 + TOK], yq_c[:TOK])
        yac = _quant_rows_into(ya[t0:t0 + TOK], yq_c[TOK:])
        aux = np.empty(AUX, np.float32)
        aux[A_YSC:A_YSC + TOK] = ysc
        aux[A_YAC:A_YAC + TOK] = yac
        aux[A_H:A_H + TOK] = Hent[t0:t0 + TOK]
        aux[A_W1S:A_W1S + D] = w1s
        aux[A_W2S:A_W2S + D] = w2s
        aux[A_B1:A_B1 + D] = b1f
        aux[A_B2:A_B2 + D] = b2f
        in_maps.append({"yq": yq_c, "wq": wq, "aux": aux})
    return in_maps, ys, ya


def _run(in_maps, trace=False):
    from concourse.bass_utils import run_bass_kernel_spmd
    nc = _get_nc()
    return run_bass_kernel_spmd(
        nc, in_maps, core_ids=list(range(N_CORES)), trace=trace
    )


def _recon(g8, ys, ya):
    # out = ya + (g8/255)*(ys-ya), fused on jax-cpu (multithreaded, one pass)
    import jax

    if "recon" not in _NC_CACHE:
        import jax.numpy as jnp

        @jax.jit
        def f(g8, ys, ya):
            g = g8.astype(jnp.float32) * np.float32(1.0 / 255.0)
            return ya + g * (ys - ya)

        _NC_CACHE["recon"] = f
    cpu = jax.devices("cpu")[0]
    with jax.default_device(cpu):
        out = _NC_CACHE["recon"](g8, ys, ya)
    return np.asarray(out)


def kernel(y_ssm, y_attn, x, W1, b1, W2, b2):
    in_maps, ys, ya = _make_in_maps(y_ssm, y_attn, x, W1, b1, W2, b2)
    res = _run(in_maps, trace=False)
    g = np.concatenate([r["g8"] for r in res.results], axis=0)  # (T, D) u8
    return _recon(g, ys, ya).reshape(B, S, D)


# revision 9
# speedup vs baseline: 4.7234x; 1.5344x over previous
"""EntropyGate fused kernel for Trainium2 NeuronCores — transfer-optimized.

Problem (hardcoded shapes): B=4, S=4096, D=2048, window=8.
  H = entropy of softmax over sliding causal window (8) of token L2 norms of x
  gate_in = [y_ssm | y_attn | H]  (B,S,2D+1)
  h = silu(gate_in @ W1 + b1); g = sigmoid(h @ W2 + b2)
  out = g*y_ssm + (1-g)*y_attn

The axon tunnel to the cores moves ~35-40 MB/s, so wall time is transfer
bound; the design minimizes bytes on the wire:
  - y_ssm/y_attn are sent as per-token-scaled int8 (1 B/elem).
  - W1/W2 are sent as per-column-scaled int8; the column scales fold into
    the Silu/Sigmoid activation `scale` operand on device (exact).
  - The entropy feature H is computed on host (needs only token norms)
    and shipped as TOK floats per core.
  - The device returns only the gate g quantized to uint8; the host
    reconstructs out = ya + g*(ys-ya) from its full-precision inputs.
Device-side, activations arrive token-major and are PE-transposed to
feature-major for the matmuls (host transposes would dominate wall time).

Sharding: tokens (B*S = 16384) split evenly across cores; weights
replicated per core.
"""

import numpy as np
import ml_dtypes

P = 128
B, S, D = 4, 4096, 2048
T = B * S                 # total tokens
N_CORES = 2               # cores used (token shards)
TOK = T // N_CORES        # tokens per core
CH = 1024                 # token chunk processed per pass
NCH = TOK // CH
MT = D // P               # 16 output blocks of 128
KC = 2 * D // P           # 32 contraction tiles for mm1
K2 = D // P               # 16 contraction tiles for mm2
WIN = 8
W2R = 2 * D + 1           # row offset of W2 inside packed wq

# aux (f32) packing offsets, per core
A_YSC = 0
A_YAC = TOK
A_H = 2 * TOK
A_W1S = 3 * TOK
A_W2S = 3 * TOK + D
A_B1 = 3 * TOK + 2 * D
A_B2 = 3 * TOK + 3 * D
AUX = 3 * TOK + 4 * D

_BF16 = ml_dtypes.bfloat16
_NC_CACHE = {}


def _ensure_jax_cache():
    # run_bass_kernel_spmd re-jits its wrapper every call; a persistent
    # compilation cache turns that (and fresh-process recompiles) into
    # fast disk hits.
    if "jaxcache" in _NC_CACHE:
        return
    import jax

    try:
        jax.config.update("jax_compilation_cache_dir", "/tmp/eg_jax_cache")
        jax.config.update("jax_persistent_cache_min_compile_time_secs", 0.5)
    except Exception:
        pass
    _NC_CACHE["jaxcache"] = True


def _build_nc():
    import concourse.bass as bass
    import concourse.tile as tile
    import concourse.mybir as mybir
    from concourse import bacc
    from contextlib import ExitStack

    f32 = mybir.dt.float32
    bf16 = mybir.dt.bfloat16
    i8 = mybir.dt.int8
    u8 = mybir.dt.uint8
    i32 = mybir.dt.int32
    AF = mybir.ActivationFunctionType
    ALU = mybir.AluOpType

    nc = bacc.Bacc("TRN2", target_bir_lowering=False, debug=False, num_devices=1)

    yq = nc.dram_tensor("yq", [2 * TOK, D], i8, kind="ExternalInput")
    wq = nc.dram_tensor("wq", [W2R + D, D], i8, kind="ExternalInput")
    aux = nc.dram_tensor("aux", [AUX], f32, kind="ExternalInput")
    g8 = nc.dram_tensor("g8", [TOK, D], u8, kind="ExternalOutput")

    with tile.TileContext(nc) as tc:
        with ExitStack() as ctx:
            const = ctx.enter_context(tc.tile_pool(name="const", bufs=1))
            stage = ctx.enter_context(tc.tile_pool(name="stage", bufs=3))
            gatep = ctx.enter_context(tc.tile_pool(name="gatep", bufs=1))
            htp = ctx.enter_context(tc.tile_pool(name="htp", bufs=1))
            gbp = ctx.enter_context(tc.tile_pool(name="gbp", bufs=1))
            gnp = ctx.enter_context(tc.tile_pool(name="gnp", bufs=2))
            wp = ctx.enter_context(tc.tile_pool(name="wp", bufs=4))
            zp = ctx.enter_context(tc.tile_pool(name="zp", bufs=3))
            smol = ctx.enter_context(tc.tile_pool(name="smol", bufs=2))
            ps = ctx.enter_context(tc.tile_pool(name="ps", bufs=8, space="PSUM"))

            # 128x128 identity for PE transposes
            iot = const.tile([P, P], i32)
            nc.gpsimd.iota(iot[:], pattern=[[1, P]], channel_multiplier=-1)
            ident = const.tile([P, P], bf16)
            nc.vector.tensor_scalar(
                ident[:], iot[:], 0, 1.0, op0=ALU.is_equal, op1=ALU.mult
            )

            # per-partition scale/bias columns: col m holds values for
            # output block m (w1scb[p, m] = w1sc[m*128 + p], etc.)
            w1scb = const.tile([P, MT], f32)
            nc.gpsimd.dma_start(w1scb[:], bass.AP(aux, A_W1S, [[1, P], [P, MT]]))
            w2scb = const.tile([P, MT], f32)
            nc.gpsimd.dma_start(w2scb[:], bass.AP(aux, A_W2S, [[1, P], [P, MT]]))
            b1sb = const.tile([P, MT], f32)
            nc.gpsimd.dma_start(b1sb[:], bass.AP(aux, A_B1, [[1, P], [P, MT]]))
            b2sb = const.tile([P, MT], f32)
            nc.gpsimd.dma_start(b2sb[:], bass.AP(aux, A_B2, [[1, P], [P, MT]]))

            NB = CH // P   # 128-token blocks per chunk
            N2 = CH // 512  # psum n-splits per chunk

            for c in range(NCH):
                # ---- phase A: load int8 y, dequant, PE-transpose to
                # feature-major gateT tiles [128 feat, CH tok] ----
                gts = []
                for y in range(2):
                    row = [gatep.tile([P, CH], bf16, name="gt",
                                      tag=f"gt{y}_{f}") for f in range(MT)]
                    gts.append(row)
                    scoff = A_YSC if y == 0 else A_YAC
                    for r in range(NB):
                        row0 = y * TOK + c * CH + r * P
                        yt = stage.tile([P, D], i8, name="yt", tag="yt")
                        nc.sync.dma_start(yt[:], yq.ap()[row0:row0 + P, :])
                        sct = smol.tile([P, 1], f32, name="sct", tag="sct")
                        nc.gpsimd.dma_start(
                            sct[:],
                            bass.AP(aux, scoff + c * CH + r * P, [[1, P], [1, 1]]),
                        )
                        dq = stage.tile([P, D], bf16, name="dq", tag="dq")
                        nc.scalar.activation(dq[:], yt[:], AF.Copy,
                                             scale=sct[:, 0:1])
                        for f in range(MT):
                            pt = ps.tile([P, P], bf16, name="ptr", tag="pt")
                            nc.tensor.transpose(
                                pt[:], dq[:, f * P:(f + 1) * P], ident[:]
                            )
                            nc.vector.tensor_copy(
                                gts[y][f][:, r * P:(r + 1) * P], pt[:]
                            )
                gflat = gts[0] + gts[1]

                # H feature row for this chunk (f32 -> bf16 on device)
                hrf = smol.tile([1, CH], f32, name="hrf", tag="hrf")
                nc.gpsimd.dma_start(
                    hrf[:], bass.AP(aux, A_H + c * CH, [[CH, 1], [1, CH]])
                )
                hrow = smol.tile([1, CH], bf16, name="hrow", tag="hrow")
                nc.scalar.activation(hrow[:], hrf[:], AF.Copy)

                # ---- mm1: hT[m, tok] = silu(s1[m]*(W1raw.T @ gateT) + b1) ----
                hts = [htp.tile([P, CH], bf16, name="ht", tag=f"ht{m}")
                       for m in range(MT)]
                for mg in range(4):
                    csl = slice(mg * 512, (mg + 1) * 512)
                    pts = [[ps.tile([P, 512], f32, name="pt1", tag="pt")
                            for _ in range(N2)] for _ in range(4)]
                    wH8 = wp.tile([1, 512], i8, name="wH8", tag="wH8")
                    nc.sync.dma_start(wH8[:], wq.ap()[2 * D:2 * D + 1, csl])
                    wH = wp.tile([1, 512], bf16, name="wH", tag="wH")
                    nc.scalar.activation(wH[:], wH8[:], AF.Copy)
                    for k in range(KC):
                        w8 = wp.tile([P, 512], i8, name="w8", tag="w8")
                        nc.sync.dma_start(w8[:], wq.ap()[k * P:(k + 1) * P, csl])
                        wb = wp.tile([P, 512], bf16, name="wb", tag="wb")
                        nc.scalar.activation(wb[:], w8[:], AF.Copy)
                        for mi in range(4):
                            for n in range(N2):
                                nc.tensor.matmul(
                                    pts[mi][n][:],
                                    wb[:, mi * P:(mi + 1) * P],
                                    gflat[k][:, n * 512:(n + 1) * 512],
                                    start=(k == 0), stop=False,
                                )
                    for mi in range(4):
                        m = mg * 4 + mi
                        for n in range(N2):
                            nc.tensor.matmul(
                                pts[mi][n][:],
                                wH[:, mi * P:(mi + 1) * P],
                                hrow[:, n * 512:(n + 1) * 512],
                                start=False, stop=True,
                            )
                            # silu(z) = z * sigmoid(z), z = s1[m]*psum + b1[m]
                            zt = zp.tile([P, 512], f32, name="zt", tag="zt")
                            nc.scalar.activation(
                                zt[:], pts[mi][n][:], AF.Identity,
                                bias=b1sb[:, m:m + 1], scale=w1scb[:, m:m + 1],
                            )
                            sg = zp.tile([P, 512], f32, name="sg", tag="sg")
                            nc.scalar.activation(sg[:], zt[:], AF.Sigmoid)
                            nc.vector.tensor_mul(
                                hts[m][:, n * 512:(n + 1) * 512], zt[:], sg[:]
                            )

                # ---- mm2: gT[e, tok] = sigmoid(s2[e]*(W2raw.T @ hT) + b2) ----
                gbs = [gbp.tile([P, CH], bf16, name="gb", tag=f"gb{e}")
                       for e in range(MT)]
                for eg in range(4):
                    esl = slice(eg * 512, (eg + 1) * 512)
                    pts2 = [[ps.tile([P, 512], f32, name="pt2", tag="pt")
                             for _ in range(N2)] for _ in range(4)]
                    for k2 in range(K2):
                        w28 = wp.tile([P, 512], i8, name="w28", tag="w8")
                        nc.sync.dma_start(
                            w28[:], wq.ap()[W2R + k2 * P:W2R + (k2 + 1) * P, esl]
                        )
                        w2b = wp.tile([P, 512], bf16, name="w2b", tag="wb")
                        nc.scalar.activation(w2b[:], w28[:], AF.Copy)
                        for ei in range(4):
                            for n in range(N2):
                                nc.tensor.matmul(
                                    pts2[ei][n][:],
                                    w2b[:, ei * P:(ei + 1) * P],
                                    hts[k2][:, n * 512:(n + 1) * 512],
                                    start=(k2 == 0), stop=(k2 == K2 - 1),
                                )
                    for ei in range(4):
                        e = eg * 4 + ei
                        for n in range(N2):
                            nc.scalar.activation(
                                gbs[e][:, n * 512:(n + 1) * 512],
                                pts2[ei][n][:], AF.Sigmoid,
                                bias=b2sb[:, e:e + 1], scale=w2scb[:, e:e + 1],
                            )

                # ---- phase D: transpose g back to token-major, quantize
                # to u8 (conversion truncates, +0.5 rounds), store ----
                for r in range(NB):
                    gn = gnp.tile([P, D], u8, name="gn", tag="gn")
                    for e in range(MT):
                        ptg = ps.tile([P, P], bf16, name="ptg", tag="pt")
                        nc.tensor.transpose(
                            ptg[:], gbs[e][:, r * P:(r + 1) * P], ident[:]
                        )
                        nc.vector.tensor_scalar(
                            gn[:, e * P:(e + 1) * P], ptg[:], 255.0, 0.5,
                            op0=ALU.mult, op1=ALU.add,
                        )
                    nc.sync.dma_start(
                        g8.ap()[c * CH + r * P:c * CH + (r + 1) * P, :], gn[:]
                    )
    nc.finalize()
    return nc


def _get_nc():
    if "nc" not in _NC_CACHE:
        _NC_CACHE["nc"] = _build_nc()
    return _NC_CACHE["nc"]


def _entropy_host(x2d):
    # token L2 norms -> sliding causal window softmax entropy, (T,) f32
    m = np.sqrt(np.einsum("sd,sd->s", x2d, x2d)).reshape(B, S)
    off = np.arange(WIN) - (WIN - 1)
    idx = np.arange(S)[:, None] + off[None, :]
    valid = idx >= 0
    idxc = np.clip(idx, 0, S - 1)
    wins = m[:, idxc]
    wins = np.where(valid[None], wins, -np.inf)
    wmax = wins.max(-1, keepdims=True)
    e = np.exp(wins - wmax)
    p = e / e.sum(-1, keepdims=True)
    H = -(p * np.log2(p + 1e-9)).sum(-1)
    return np.ascontiguousarray(H.reshape(-1).astype(np.float32))


def _quant_rows_into(a, out_i8):
    # per-row symmetric int8: returns scales (rows,) f32
    hi = a.max(axis=1)
    lo = a.min(axis=1)
    s = np.maximum(hi, -lo)
    s /= 127.0
    np.maximum(s, 1e-30, out=s)
    inv = 1.0 / s
    tmp = a * inv[:, None]
    np.rint(tmp, out=tmp)
    out_i8[:] = tmp
    return s.astype(np.float32)


def _quant_cols(w):
    # per-column symmetric int8: returns (q, scales (cols,) f32)
    aw = np.abs(w).max(axis=0)
    s = np.maximum(aw / 127.0, 1e-30).astype(np.float32)
    tmp = w * (1.0 / s)[None, :]
    np.rint(tmp, out=tmp)
    return tmp.astype(np.int8), s


def _make_in_maps(y_ssm, y_attn, x, W1, b1, W2, b2):
    ys = np.asarray(y_ssm, np.float32).reshape(T, D)
    ya = np.asarray(y_attn, np.float32).reshape(T, D)
    xs = np.asarray(x, np.float32).reshape(T, D)
    W1f = np.asarray(W1, np.float32)
    W2f = np.asarray(W2, np.float32)
    b1f = np.asarray(b1, np.float32)
    b2f = np.asarray(b2, np.float32)

    Hent = _entropy_host(xs)

    w1q, w1s = _quant_cols(W1f)
    w2q, w2s = _quant_cols(W2f)
    wq = np.concatenate([w1q, w2q], axis=0)  # (2D+1+D, D): W2 rows start at W2R
    assert wq.shape[0] == W2R + D

    in_maps = []
    for c in range(N_CORES):
        t0 = c * TOK
        yq_c = np.empty((2 * TOK, D), np.int8)
        ysc = _quant_rows_into(ys[t0:t0 + TOK], yq_c[:TOK])
        yac = _quant_rows_into(ya[t0:t0 + TOK], yq_c[TOK:])
        aux = np.empty(AUX, np.float32)
        aux[A_YSC:A_YSC + TOK] = ysc
        aux[A_YAC:A_YAC + TOK] = yac
        aux[A_H:A_H + TOK] = Hent[t0:t0 + TOK]
        aux[A_W1S:A_W1S + D] = w1s
        aux[A_W2S:A_W2S + D] = w2s
        aux[A_B1:A_B1 + D] = b1f
        aux[A_B2:A_B2 + D] = b2f
        in_maps.append({"yq": yq_c, "wq": wq, "aux": aux})
    return in_maps, ys, ya


def _run(in_maps, trace=False):
    from concourse.bass_utils import run_bass_kernel_spmd
    _ensure_jax_cache()
    nc = _get_nc()
    return run_bass_kernel_spmd(
        nc, in_maps, core_ids=list(range(N_CORES)), trace=trace
    )


def _recon(g8, ys, ya):
    # out = ya + (g8/255)*(ys-ya), fused on jax-cpu (multithreaded, one pass)
    import jax

    if "recon" not in _NC_CACHE:
        import jax.numpy as jnp

        @jax.jit
        def f(g8, ys, ya):
            g = g8.astype(jnp.float32) * np.float32(1.0 / 255.0)
            return ya + g * (ys - ya)

        _NC_CACHE["recon"] = f
    cpu = jax.devices("cpu")[0]
    with jax.default_device(cpu):
        out = _NC_CACHE["recon"](g8, ys, ya)
    return np.asarray(out)


def kernel(y_ssm, y_attn, x, W1, b1, W2, b2):
    in_maps, ys, ya = _make_in_maps(y_ssm, y_attn, x, W1, b1, W2, b2)
    res = _run(in_maps, trace=False)
    g = np.concatenate([r["g8"] for r in res.results], axis=0)  # (T, D) u8
    return _recon(g, ys, ya).reshape(B, S, D)
